# revision 9
# baseline (speedup 1.0000x reference)
"""EnvironmentConsistentAttention on 8 trn2 cores.

Sharding: 4 images x 2 directions = 8 independent units, one per core (the
horizontal direction of image x equals the vertical direction of x spatially
transposed). Per core: given shifted maps A, B [31,32,256] compute
(yA, yB) = _corr_recon(A, B), each emitted channel-major [256, 992].

Key algebra (Hp=31, Wp=32, C=256, L=992, padded grid 33x34=1122):
  z[(p,q,c), j] = A_pad[j+(p,q), c] * B_pad[j+(p,q), c] = W_pad[j+(p,q), c]
with W = A*B elementwise. Hence the patch gram factorizes:
  R[j,i] = z.T@z [j,i] = sum_d Q[j+d, i+d],  Q = W_pad @ W_pad.T  (C=256 contraction)
i.e. R is the 3x3 *diagonal box-sum* of the tiny gram Q. Similarly the
conv-transpose reconstruction collapses to a single matmul against the
box-sum of the softmax matrix:
  ya[l',c] = sum_m A_pad[m,c] * G[m,l'],   G = sum_d shift_{d,d}(S.T)
So the kernel is: Q (PE) -> boxsum (DMA-staged shifted adds, DVE+Pool) ->
exp (Act, per-partition bias kills padded border rows) -> denominators
(ones-matmul) -> S.T -> boxsum -> recon matmul (PE). Everything bf16 except
PSUM accumulation; inverse patch norms are host-precomputed and folded into
the exp scale / a broadcast multiplier.

Layout: all big intermediates live on a padded-grid partition axis
(1122 rows -> 9 chunks of 128, side by side in one wide SBUF tile so the
partition-shifted staging copies are 2 contiguous DMAs per direction per
3-chunk group). Free axis per chunk: 36 guard | 1122 padded grid | 36 guard.
"""

import numpy as np

Hp, Wp, C = 31, 32, 256
L = Hp * Wp              # 992
PH, PW = 33, 34
NPAD = PH * PW           # 1122
NCH = 9                  # partition chunks of 128 (1152 rows, tail junk)
GL = 36                  # free-dim guard cols per side
WB = GL + NPAD + GL      # 1194 block width
WBIG = NCH * WB          # 10746
B_IMG, H_IMG, W_IMG = 4, 32, 32
SOFTMAX_SCALE = 10.0

_CACHE = {}


def _build_program():
    import concourse.bass as bass
    import concourse.tile as tile
    from concourse import bacc, mybir
    from contextlib import ExitStack

    f32 = mybir.dt.float32
    bf16 = mybir.dt.bfloat16
    Exp = mybir.ActivationFunctionType.Exp

    nc = bacc.Bacc("TRN2", target_bir_lowering=False, debug=False)

    w_chw = nc.dram_tensor("w_chw", [C, NPAD], bf16, kind="ExternalInput")
    ab_m = nc.dram_tensor("ab_m", [NCH * 128, 2 * C], bf16, kind="ExternalInput")
    inv_b = nc.dram_tensor("inv_b", [1, NPAD], bf16, kind="ExternalInput")
    maskD = nc.dram_tensor("maskD", [1, NPAD], f32, kind="ExternalInput")
    exp_sc = nc.dram_tensor("exp_sc", [128, NCH], f32, kind="ExternalInput")
    exp_bs = nc.dram_tensor("exp_bs", [128, NCH], f32, kind="ExternalInput")
    ya_t = nc.dram_tensor("ya_t", [C, L], bf16, kind="ExternalOutput")
    yb_t = nc.dram_tensor("yb_t", [C, L], bf16, kind="ExternalOutput")

    # free-dim col of padded position n within block b: WB*b + GL + n
    def c0(b):
        return WB * b

    with tile.TileContext(nc) as tc:
        with ExitStack() as ctx:
            const = ctx.enter_context(tc.tile_pool(name="const", bufs=1))
            stgp = {
                k: ctx.enter_context(tc.tile_pool(name=f"stg_{k}", bufs=2))
                for k in ("sp1", "sm1", "sp34", "sm34")
            }
            gp = ctx.enter_context(tc.tile_pool(name="gp", bufs=4))
            outp = ctx.enter_context(tc.tile_pool(name="outp", bufs=4))

            # persistent big tiles
            QE = const.tile([128, WBIG], bf16, tag="QE")      # Q, then E/S.T in place
            T1 = const.tile([128, WBIG], bf16, tag="T1")      # w-pass accum (both boxsums)
            abm = const.tile([128, NCH * 2 * C], bf16, tag="abm")
            wg = [
                const.tile([128, 1152], bf16, tag=f"wg{i}", name=f"wg{i}")
                for i in range(2)
            ]
            invb = const.tile([128, NPAD], bf16, tag="invb")
            rb = const.tile([128, NPAD], bf16, tag="rb")
            sct = const.tile([128, NCH], f32, tag="sct")
            bst = const.tile([128, NCH], f32, tag="bst")
            mskt = const.tile([1, NPAD], f32, tag="mskt")
            rcp = const.tile([1, NPAD], f32, tag="rcp")
            rcpb = const.tile([1, NPAD], bf16, tag="rcpb")
            ones_k = const.tile([128, 1], bf16, tag="onesk")
            ones_m = const.tile([1, 128], bf16, tag="onesm")
            zt = const.tile([34, WB], bf16, tag="zt")

            # ---- loads + one-time zeroing ----
            for i in range(2):
                nc.sync.dma_start(out=wg[i][:, 0:NPAD], in_=w_chw[128 * i : 128 * (i + 1), :])
                nc.vector.memset(wg[i][:, NPAD:1152], 0.0)
            abm_src = bass.AP(
                tensor=ab_m.ap().tensor,
                offset=0,
                ap=[[2 * C, 128], [2 * C * 128, NCH], [1, 2 * C]],
            )
            nc.sync.dma_start(out=abm[:], in_=abm_src)
            nc.sync.dma_start(out=invb[:], in_=inv_b.ap().to_broadcast([128, NPAD]))
            nc.sync.dma_start(out=sct[:], in_=exp_sc[:, :])
            nc.sync.dma_start(out=bst[:], in_=exp_bs[:, :])
            nc.sync.dma_start(out=mskt[:], in_=maskD[:, :])
            nc.vector.memset(ones_k[:], 1.0)
            nc.vector.memset(ones_m[:], 1.0)
            nc.gpsimd.memset(zt[:], 0.0)
            for b in range(NCH):
                nc.gpsimd.memset(QE[:, c0(b) : c0(b) + GL], 0.0)
                nc.gpsimd.memset(QE[:, c0(b) + GL + NPAD : c0(b) + WB], 0.0)
                nc.vector.memset(T1[:, c0(b) : c0(b) + 2], 0.0)
                nc.vector.memset(T1[:, c0(b) + WB - 2 : c0(b) + WB], 0.0)

            # ---- stage helpers: partition-shifted copies of 3-block groups ----
            # stage[p, blk-local] = X_global[128*blk + p + delta]
            def stage_w(X, g, tag):
                """delta = +-1 partition shifts for group g (blocks 3g..3g+3)."""
                lo, hi = 3 * g * WB, (3 * g + 3) * WB
                sp = stgp["sp1"].tile([128, 3 * WB], bf16, tag="sp1")
                nc.sync.dma_start(out=sp[0:127, :], in_=X[1:128, lo:hi])
                if g < 2:
                    nc.sync.dma_start(out=sp[127:128, :], in_=X[0:1, lo + WB : hi + WB])
                else:
                    nc.sync.dma_start(out=sp[127:128, 0 : 2 * WB], in_=X[0:1, lo + WB : hi])
                    nc.sync.dma_start(out=sp[127:128, 2 * WB : 3 * WB], in_=zt[0:1, :])
                sm = stgp["sm1"].tile([128, 3 * WB], bf16, tag="sm1")
                nc.sync.dma_start(out=sm[1:128, :], in_=X[0:127, lo:hi])
                if g > 0:
                    nc.sync.dma_start(out=sm[0:1, :], in_=X[127:128, lo - WB : hi - WB])
                else:
                    nc.sync.dma_start(out=sm[0:1, WB : 3 * WB], in_=X[127:128, 0 : 2 * WB])
                    nc.sync.dma_start(out=sm[0:1, 0:WB], in_=zt[0:1, :])
                return sp, sm

            def stage_h(X, g, tag):
                """delta = +-34 partition shifts for group g."""
                lo, hi = 3 * g * WB, (3 * g + 3) * WB
                sp = stgp["sp34"].tile([128, 3 * WB], bf16, tag="sp34")
                nc.scalar.dma_start(out=sp[0:94, :], in_=X[34:128, lo:hi])
                if g < 2:
                    nc.scalar.dma_start(out=sp[94:128, :], in_=X[0:34, lo + WB : hi + WB])
                else:
                    nc.scalar.dma_start(out=sp[94:128, 0 : 2 * WB], in_=X[0:34, lo + WB : hi])
                    nc.scalar.dma_start(out=sp[94:128, 2 * WB : 3 * WB], in_=zt[0:34, :])
                sm = stgp["sm34"].tile([128, 3 * WB], bf16, tag="sm34")
                nc.gpsimd.dma_start(out=sm[34:128, :], in_=X[0:94, lo:hi])
                if g > 0:
                    nc.gpsimd.dma_start(out=sm[0:34, :], in_=X[94:128, lo - WB : hi - WB])
                else:
                    nc.gpsimd.dma_start(out=sm[0:34, WB : 3 * WB], in_=X[94:128, 0 : 2 * WB])
                    nc.gpsimd.dma_start(out=sm[0:34, 0:WB], in_=zt[0:34, :])
                return sp, sm

            # w-pass adds (DVE): T1[x] = sm[x-1] + X[x] + sp[x+1], x in [2, WB-2)
            def wadd(X, b, sp, sm):
                bl = (b % 3) * WB
                o = T1[:, c0(b) + 2 : c0(b) + WB - 2]
                nc.vector.tensor_add(o, sm[:, bl + 1 : bl + WB - 3], X[:, c0(b) + 2 : c0(b) + WB - 2])
                nc.vector.tensor_add(o, o, sp[:, bl + 3 : bl + WB - 1])

            # h-pass adds (Pool): OUT[x] = sm[x-34] + T1[x] + sp[x+34], x in [GL, GL+NPAD)
            def hadd(OUT, b, sp, sm, obase=None):
                bl = (b % 3) * WB
                ob = c0(b) if obase is None else obase
                o = OUT[:, ob + GL : ob + GL + NPAD]
                nc.gpsimd.tensor_add(
                    o, sm[:, bl + GL - 34 : bl + GL + NPAD - 34], T1[:, c0(b) + GL : c0(b) + GL + NPAD]
                )
                nc.gpsimd.tensor_add(o, o, sp[:, bl + GL + 34 : bl + GL + NPAD + 34])

            # ================= phase 1: Q, boxsum -> R, exp, denominators ======
            with ExitStack() as ph1:
                qps = ph1.enter_context(tc.tile_pool(name="qps", bufs=1, space="PSUM"))
                dpsp = ph1.enter_context(tc.tile_pool(name="dps", bufs=1, space="PSUM"))
                dps = dpsp.tile([1, NPAD], f32, tag="dps", name="dps")
                SPLITS = [(0, 512), (512, 512), (1024, 98)]

                def qmm(b):
                    q = qps.tile([128, NPAD], f32, tag="qps", name=f"q{b}")
                    for s, n in SPLITS:
                        for cc in range(2):
                            nc.tensor.matmul(
                                q[:, s : s + n],
                                wg[cc][:, 128 * b : 128 * b + 128],
                                wg[cc][:, s : s + n],
                                start=(cc == 0),
                                stop=(cc == 1),
                            )
                        # per-split drain so the single psum buffer pipelines
                        nc.scalar.copy(
                            QE[:, c0(b) + GL + s : c0(b) + GL + s + n], q[:, s : s + n]
                        )

                def scale_exp_denom(b):
                    r = QE[:, c0(b) + GL : c0(b) + GL + NPAD]
                    nc.vector.tensor_mul(r, r, invb[:, :])
                    nc.scalar.activation(
                        r, r, Exp, bias=bst[:, b : b + 1], scale=sct[:, b : b + 1]
                    )
                    for s, n in SPLITS:
                        nc.tensor.matmul(
                            dps[:, s : s + n],
                            ones_k[:, :],
                            QE[:, c0(b) + GL + s : c0(b) + GL + s + n],
                            start=(b == 0),
                            stop=(b == NCH - 1),
                        )

                for b in range(6):
                    qmm(b)
                ws = {}
                ws[0] = stage_w(QE, 0, "q")
                for b in range(3):
                    wadd(QE, b, *ws[0])
                for b in range(6, 9):
                    qmm(b)
                ws[1] = stage_w(QE, 1, "q")
                for b in range(3, 6):
                    wadd(QE, b, *ws[1])
                ws[2] = stage_w(QE, 2, "q")
                for b in range(6, 9):
                    wadd(QE, b, *ws[2])
                for g in range(3):
                    hs = stage_h(T1, g, "t")
                    for b in range(3 * g, 3 * g + 3):
                        hadd(QE, b, *hs)   # R overwrites Q in QE
                    for b in range(3 * g, 3 * g + 3):
                        scale_exp_denom(b)

                # ---- softmax denominators -> broadcast reciprocal ----
                nc.vector.reciprocal_approx_fast(out=rcp[:, :], in_=dps[:, :])
                nc.vector.tensor_mul(rcpb[:, :], rcp[:, :], mskt[:, :])

            with ExitStack() as phb:
                bps = phb.enter_context(tc.tile_pool(name="bps", bufs=1, space="PSUM"))
                bp = bps.tile([128, NPAD], f32, tag="bps", name="bp")
                for s, n in [(0, 512), (512, 512), (1024, 98)]:
                    nc.tensor.matmul(
                        bp[:, s : s + n], ones_m[:, :], rcpb[:, s : s + n],
                        start=True, stop=True,
                    )
                nc.scalar.copy(rb[:, :], bp[:, :])

            # ================= phase 2: S.T, boxsum -> G, recon ================
            with ExitStack() as ph2:
                yps = ph2.enter_context(tc.tile_pool(name="yps", bufs=8, space="PSUM"))
                HALVES = [(0, 16, 512), (16, 15, 480)]
                yp = [
                    [
                        yps.tile([128, n], f32, tag="yps", name=f"y{cb}_{hi}")
                        for hi, (h0, nh, n) in enumerate(HALVES)
                    ]
                    for cb in range(4)
                ]

                def stmult(b):
                    e = QE[:, c0(b) + GL : c0(b) + GL + NPAD]
                    nc.vector.tensor_mul(e, e, rb[:, :])

                def recon(b, G):
                    gv = G[:, GL : GL + NPAD].rearrange("p (H W) -> p H W", H=PH, W=PW)
                    for cb in range(4):
                        lhs = abm[:, 512 * b + 128 * cb : 512 * b + 128 * (cb + 1)]
                        for hi, (h0, nh, n) in enumerate(HALVES):
                            nc.tensor.matmul(
                                yp[cb][hi][:, :],
                                lhs,
                                gv[:, 1 + h0 : 1 + h0 + nh, 1 : 1 + Wp],
                                start=(b == 0),
                                stop=(b == NCH - 1),
                            )

                for b in range(6):
                    stmult(b)
                w2 = {}
                w2[0] = stage_w(QE, 0, "s")
                for b in range(3):
                    wadd(QE, b, *w2[0])
                for b in range(6, 9):
                    stmult(b)
                w2[1] = stage_w(QE, 1, "s")
                for b in range(3, 6):
                    wadd(QE, b, *w2[1])
                w2[2] = stage_w(QE, 2, "s")
                for b in range(6, 9):
                    wadd(QE, b, *w2[2])
                for g in range(3):
                    hs = stage_h(T1, g, "t2")
                    for b in range(3 * g, 3 * g + 3):
                        G = gp.tile([128, WB], bf16, tag="g", name=f"g{b}")
                        hadd(G, b, *hs, obase=0)
                        recon(b, G)

                for cb, dram in ((0, ya_t), (1, ya_t), (2, yb_t), (3, yb_t)):
                    ot = outp.tile([128, L], bf16, tag="ot", name=f"ot{cb}")
                    off = 0
                    for hi, (h0, nh, n) in enumerate(HALVES):
                        nc.scalar.copy(ot[:, off : off + n], yp[cb][hi][:, :])
                        off += n
                    r0 = 128 * (cb % 2)
                    nc.sync.dma_start(out=dram[r0 : r0 + 128, :], in_=ot[:])

    nc.compile()
    return nc


def _get_program():
    if "nc" not in _CACHE:
        _CACHE["nc"] = _build_program()
    return _CACHE["nc"]


def _core_inputs(A, B):
    """A, B: [31,32,256] float32 -> per-core input map."""
    import ml_dtypes

    BF = np.dtype(ml_dtypes.bfloat16)
    ap = np.zeros((PH, PW, C), np.float32)
    ap[1 : 1 + Hp, 1 : 1 + Wp] = A
    bp = np.zeros((PH, PW, C), np.float32)
    bp[1 : 1 + Hp, 1 : 1 + Wp] = B

    def inv_norm(pad):
        s = (pad.astype(np.float64) ** 2).sum(-1)  # [33,34]
        ss = np.zeros((Hp, Wp))
        for p in range(3):
            for q in range(3):
                ss += s[p : p + Hp, q : q + Wp]
        return 1.0 / np.maximum(np.sqrt(ss), 1e-4)

    inv = inv_norm(ap) * inv_norm(bp)          # [31, 32]
    invp = np.zeros((PH, PW))
    invp[1 : 1 + Hp, 1 : 1 + Wp] = inv          # padded, zero borders
    invf = invp.reshape(-1)                     # [1122]

    w = (ap * bp).reshape(NPAD, C)
    abm = np.zeros((NCH * 128, 2 * C), np.float32)
    abm[:NPAD, :C] = ap.reshape(NPAD, C)
    abm[:NPAD, C:] = bp.reshape(NPAD, C)

    interior = np.zeros(NCH * 128, np.float32)
    interior[:NPAD] = (invf > 0).astype(np.float32)
    sc = (SOFTMAX_SCALE * np.pad(invf, (0, NCH * 128 - NPAD))).astype(np.float32)
    bs = np.where(interior > 0, 0.0, -80.0).astype(np.float32)

    return {
        "w_chw": np.ascontiguousarray(w.T).astype(BF),
        "ab_m": abm.astype(BF),
        "inv_b": invf.reshape(1, NPAD).astype(BF),
        "maskD": interior[:NPAD].reshape(1, NPAD).astype(np.float32),
        "exp_sc": np.ascontiguousarray(sc.reshape(NCH, 128).T),
        "exp_bs": np.ascontiguousarray(bs.reshape(NCH, 128).T),
    }


def _untp(y_t):
    # [256, 992] channel-major -> [31, 32, 256]
    return np.asarray(y_t).astype(np.float32).reshape(C, Hp, Wp).transpose(1, 2, 0)


def kernel(x, mask):
    x = np.asarray(x, dtype=np.float32)
    in_maps = []
    for b in range(B_IMG):
        xb = x[b]
        in_maps.append(_core_inputs(xb[:-1], xb[1:]))
        xt = np.ascontiguousarray(xb.transpose(1, 0, 2))
        in_maps.append(_core_inputs(xt[1:], xt[:-1]))

    from concourse.bass_utils import run_bass_kernel_spmd

    nc = _get_program()
    res = run_bass_kernel_spmd(nc, in_maps, list(range(8))).results

    out = np.empty((B_IMG, H_IMG, W_IMG, C), np.float32)
    for b in range(B_IMG):
        yl = _untp(res[2 * b]["ya_t"])
        yr = _untp(res[2 * b]["yb_t"])
        ylr = np.concatenate([yr[:1], (yr[1:] + yl[:-1]) * 0.5, yl[-1:]], axis=0)
        yt = _untp(res[2 * b + 1]["ya_t"]).transpose(1, 0, 2)
        yb = _untp(res[2 * b + 1]["yb_t"]).transpose(1, 0, 2)
        ytb = np.concatenate(
            [yt[:, :1], (yt[:, 1:] + yb[:, :-1]) * 0.5, yb[:, -1:]], axis=1
        )
        out[b] = (ylr + ytb) * 0.5
    return out


# revision 12
# speedup vs baseline: 2.8082x; 2.8082x over previous
"""EnvironmentConsistentAttention on 8 trn2 cores.

Sharding: 4 images x 2 directions (vertical/horizontal neighbor pairs) = 8
independent units, one per core. The horizontal direction of image x equals
the vertical direction of x spatially transposed, so a single SPMD program
handles both: given shifted maps A, B [31,32,256] it returns
(yA, yB) = _corr_recon(A, B), each [31,32,256] (emitted channel-major).

Per-core math (Hp=31, Wp=32, C=256, L=992, k=3):
  pa[(p,q,c), l=(h,w)] = A_pad[h+p, w+q, c]          (zero-padded patches)
  z = pa * pb                                        [2304, L]
  R = z.T @ z                                        [L, L] gram
  att[i,j] = inv[i]*inv[j]*R[i,j];  S = softmax(10*att, axis=j)
  yA = conv_transpose(S, pa) -> ya[l',c] = sum_{p,q,j} S[shift(l',p,q), j]*pa[(p,q,c), j]

att is symmetric pre-softmax, so tiles of R computed as [j-part, i-free] are
directly S.T tiles; exp/softmax-denominator (a cross-partition ones-matmul)
and the reconstruction all run in that transposed layout. S.T is stored in a
[33,34]-padded spatial grid over i so the 9 conv-transpose shifts become pure
access-pattern offsets (zero borders give SAME-padding semantics for free).
Patch norms are folded in as row/column scales of R (host precomputes the
tiny [992] inverse-norm vector).
"""

import numpy as np

Hp, Wp, C = 31, 32, 256
L = Hp * Wp            # 992
PH, PW = Hp + 2, Wp + 2  # 33, 34 padded grid
NPAD = PH * PW         # 1122
KK = 9 * C             # 2304
JC = [(128 * c, 128 if c < 7 else 96) for c in range(8)]   # j/l chunks
HALves = [(0, 512, 0, 16), (512, 480, 16, 15)]  # (i0, n, h0, nh) over i/l'
B_IMG, H_IMG, W_IMG = 4, 32, 32

_CACHE = {}


def _build_program():
    import concourse.bass as bass
    import concourse.tile as tile
    from concourse import bacc, mybir

    f32 = mybir.dt.float32
    f32r = mybir.dt.float32r

    def r(ap):
        return ap.bitcast(f32r)

    nc = bacc.Bacc("TRN2", target_bir_lowering=False, debug=False)

    a_pad = nc.dram_tensor("a_pad", [PH, PW, C], f32, kind="ExternalInput")
    b_pad = nc.dram_tensor("b_pad", [PH, PW, C], f32, kind="ExternalInput")
    a_chw = nc.dram_tensor("a_chw", [C, NPAD], f32, kind="ExternalInput")
    b_chw = nc.dram_tensor("b_chw", [C, NPAD], f32, kind="ExternalInput")
    inv_p = nc.dram_tensor("inv_p", [128, 8], f32, kind="ExternalInput")
    inv_f = nc.dram_tensor("inv_f", [1, L], f32, kind="ExternalInput")
    ya_t = nc.dram_tensor("ya_t", [C, L], f32, kind="ExternalOutput")
    yb_t = nc.dram_tensor("yb_t", [C, L], f32, kind="ExternalOutput")

    with tile.TileContext(nc) as tc:
        from contextlib import ExitStack

        with ExitStack() as ctx:
            const = ctx.enter_context(tc.tile_pool(name="const", bufs=1))
            outp = ctx.enter_context(tc.tile_pool(name="outp", bufs=4))
            tpadp = ctx.enter_context(tc.tile_pool(name="tpad", bufs=8))

            # Constants (input DMAs for these are emitted after the chw
            # loads so the z-build critical path gets the DMA queue first)
            sb_inv_p = const.tile([128, 8], f32, tag="invp")
            sb_inv_b = const.tile([128, L], f32, tag="invb")
            ones_f = const.tile([128, 128], f32, tag="onesf")
            nc.vector.memset(ones_f[:], 1.0)
            ones_k = const.tile([128, 1], f32r, tag="onesk")
            nc.scalar.copy(ones_k[:], ones_f[:, 0:1])
            ones_m = const.tile([1, 128], f32r, tag="onesm")
            nc.scalar.copy(ones_m[:], ones_f[0:1, :])
            from concourse.masks import make_identity

            idn_f = const.tile([128, 128], f32, tag="idnf")
            idn = const.tile([128, 128], f32r, tag="idn")
            make_identity(nc, idn_f[:])
            nc.scalar.copy(idn[:], idn_f[:])
            recip_sb = const.tile([1, L], f32r, tag="recip")
            rb_sb = const.tile([128, L], f32, tag="rbcast")

            # S.T tiles in padded-grid layout, zeroed borders
            tpad = [
                tpadp.tile([128, NPAD], f32r, tag="tpad", name=f"tpad{c}")
                for c in range(8)
            ]

            with ExitStack() as ph1:
                apadp = ph1.enter_context(tc.tile_pool(name="apad", bufs=4))
                zp = ph1.enter_context(tc.tile_pool(name="z", bufs=18))
                psD = ph1.enter_context(
                    tc.tile_pool(name="psD", bufs=1, space="PSUM")
                )

                # Load padded inputs channel-major; build z = pa*pb views
                achw, bchw = [], []
                dma_engs = [nc.sync, nc.scalar, nc.sync, nc.scalar]
                for ch in range(2):
                    ta = apadp.tile([128, NPAD], f32, tag="apad")
                    tb = apadp.tile([128, NPAD], f32, tag="apad")
                    dma_engs[2 * ch].dma_start(
                        out=ta[:], in_=a_chw[128 * ch : 128 * (ch + 1), :]
                    )
                    dma_engs[2 * ch + 1].dma_start(
                        out=tb[:], in_=b_chw[128 * ch : 128 * (ch + 1), :]
                    )
                    achw.append(ta)
                    bchw.append(tb)
                nc.sync.dma_start(out=sb_inv_p[:], in_=inv_p[:, :])
                nc.sync.dma_start(
                    out=sb_inv_b[:], in_=inv_f.ap().to_broadcast([128, L])
                )

                zt = []
                for p in range(3):
                    for q in range(3):
                        for ch in range(2):
                            k = len(zt)
                            zk = zp.tile([128, L], f32r, tag="z")
                            av = achw[ch].rearrange(
                                "c (h w) -> c h w", h=PH, w=PW
                            )[:, p : p + Hp, q : q + Wp]
                            bv = bchw[ch].rearrange(
                                "c (h w) -> c h w", h=PH, w=PW
                            )[:, p : p + Hp, q : q + Wp]
                            nc.vector.tensor_mul(zk[:], av, bv)
                            zt.append(zk)

                # zero S.T borders (gpsimd; only borders matter, interior is
                # overwritten by the exp)
                for c in range(8):
                    tf = tpad[c].bitcast(f32).rearrange(
                        "j (h w) -> j h w", h=PH, w=PW
                    )
                    nc.gpsimd.memset(tf[:, 0:1, :], 0.0)
                    nc.gpsimd.memset(tf[:, PH - 1 : PH, :], 0.0)
                    nc.gpsimd.memset(tf[:, :, 0:1], 0.0)
                    nc.gpsimd.memset(tf[:, :, PW - 1 : PW], 0.0)

                # Gram R = z.T@z per (j-chunk, i-half); scale+exp into tpad;
                # accumulate softmax denominators with ones-matmuls.
                dpsall = psD.tile([1, L], f32, tag="dps", name="dpsall")
                dps = [dpsall[:, i0 : i0 + n] for (i0, n, _, _) in HALves]
                # E is symmetric: compute only i >= 128*jc (upper block
                # triangle incl. diagonal), mirror the rest by PE transpose.
                # (i0, n, s0): matmul computes i in [i0, i0+n); only
                # [i0+s0, i0+n) is written out. All n >= 256 so f32r matmuls
                # stream at 1 cycle/row (free dims < 256 drop to 1/4 rate);
                # short tails extend left into already-covered i and skip the
                # overlap on write.
                def ichunks(jc):
                    off = 128 * jc
                    ln = L - off
                    if ln > 512:
                        n0 = ((ln + 63) // 64) * 32  # ~half, 32-aligned
                        return [(off, n0, 0), (off + n0, ln - n0, 0)]
                    if ln >= 256:
                        return [(off, ln, 0)]
                    return [(L - 256, 256, 256 - ln)]

                with tc.tile_pool(name="psR", bufs=6, space="PSUM") as psR:
                    for g0, g1 in ((0, 3), (3, 6), (6, 8)):
                        grp = list(enumerate(JC))[g0:g1]
                        rps = {
                            c: [
                                psR.tile(
                                    [128, n], f32, tag="rps", name=f"rps{c}_{ci}"
                                )
                                for ci, (i0, n, s0) in enumerate(ichunks(c))
                            ]
                            for c, _ in grp
                        }
                        # k-major so early matmuls only need early z tiles
                        for k in range(18):
                            for c, (j0, dm) in grp:
                                for ci, (i0, n, s0) in enumerate(ichunks(c)):
                                    nc.tensor.matmul(
                                        rps[c][ci][:dm, :],
                                        zt[k][:, j0 : j0 + dm],
                                        zt[k][:, i0 : i0 + n],
                                        start=(k == 0),
                                        stop=(k == 17),
                                    )
                        for c, (j0, dm) in grp:
                            t3 = tpad[c].rearrange("j (h w) -> j h w", h=PH, w=PW)
                            for ci, (i0, n, s0) in enumerate(ichunks(c)):
                                i0w, nw = i0 + s0, n - s0
                                h0, nh = i0w // Wp, nw // Wp
                                itv = t3[:dm, 1 + h0 : 1 + h0 + nh, 1 : 1 + Wp]
                                nc.vector.tensor_mul(
                                    itv,
                                    rps[c][ci][:dm, s0:n],
                                    sb_inv_b[:dm, i0w : i0w + nw],
                                )
                                nc.scalar.activation(
                                    itv,
                                    itv,
                                    mybir.ActivationFunctionType.Exp,
                                    scale=sb_inv_p[:dm, c : c + 1],
                                )

                # mirror lower-triangle blocks, then the softmax denominators
                with tc.tile_pool(name="psT", bufs=2, space="PSUM") as psT, \
                        tc.tile_pool(name="tbp", bufs=3) as tbp:
                    for c, (j0, dm) in enumerate(JC):
                        t3j = tpad[c].rearrange("j (h w) -> j h w", h=PH, w=PW)
                        nhj = dm // Wp
                        for ic in range(c):
                            t3s = tpad[ic].rearrange(
                                "j (h w) -> j h w", h=PH, w=PW
                            )
                            srcv = t3s[:128, 1 + 4 * c : 1 + 4 * c + nhj, 1 : 1 + Wp]
                            tbn = tbp.tile(
                                [128, 128], f32r, tag="tbn", name=f"tbn{c}_{ic}"
                            )
                            nc.vector.tensor_copy(tbn[:, :dm], srcv)
                            pst = psT.tile(
                                [128, 128], f32r, tag="pst", name=f"pst{c}_{ic}"
                            )
                            nc.tensor.transpose(pst[:dm, :128], tbn[:, :dm], idn[:, :])
                            nc.vector.tensor_copy(
                                t3j[:dm, 1 + 4 * ic : 1 + 4 * ic + 4, 1 : 1 + Wp],
                                pst[:dm, :128],
                            )
                        for hi, (i0, n, h0, nh) in enumerate(HALves):
                            nc.tensor.matmul(
                                dps[hi],
                                ones_k[:dm, :],
                                t3j[:dm, 1 + h0 : 1 + h0 + nh, 1 : 1 + Wp],
                                start=(c == 0),
                                stop=(c == 7),
                            )

                # 1/denom, broadcast across partitions via K=1 matmul
                rtmp2 = const.tile([1, L], f32, tag="rtmp2")
                nc.vector.reciprocal_approx_fast(out=rtmp2[:, :], in_=dpsall[:, :])
                nc.vector.tensor_copy(recip_sb[:, :], rtmp2[:, :])
                psB = ph1.enter_context(
                    tc.tile_pool(name="psB", bufs=1, space="PSUM")
                )
                bpsall = psB.tile([128, L], f32, tag="bps", name="bpsall")
                for hi, (i0, n, _, _) in enumerate(HALves):
                    nc.tensor.matmul(
                        bpsall[:, i0 : i0 + n],
                        ones_m[:, :],
                        recip_sb[:, i0 : i0 + n],
                        start=True,
                        stop=True,
                    )
                nc.scalar.copy(rb_sb[:, :], bpsall[:, :])

            # Reconstruction, a/b interleaved over one jc sweep; the
            # softmax denominator is applied to each S.T chunk at the top of
            # its jc iteration so recon matmuls chase the scaling.
            # yaT[c, l'] += sum_{p,q,j} paT[j,(p,q,c)]*S.T[j, i(l',p,q)]
            with ExitStack() as ph2:
                patp = ph2.enter_context(tc.tile_pool(name="pat", bufs=4))
                psY = ph2.enter_context(
                    tc.tile_pool(name="psY", bufs=8, space="PSUM")
                )
                yps = [
                    [
                        [
                            psY.tile(
                                [128, n], f32, tag="yps", name=f"yps{t}_{cb}_{hi}"
                            )
                            for hi, (_, n, _, _) in enumerate(HALves)
                        ]
                        for cb in range(2)
                    ]
                    for t in range(2)
                ]
                for c, (j0, dm) in enumerate(JC):
                    h0j, nhj = 4 * c, (4 if c < 7 else 3)
                    t3 = tpad[c].rearrange("j (h w) -> j h w", h=PH, w=PW)
                    for hi, (i0, n, h0, nh) in enumerate(HALves):
                        itv = t3[:dm, 1 + h0 : 1 + h0 + nh, 1 : 1 + Wp]
                        nc.vector.tensor_mul(itv, itv, rb_sb[:dm, i0 : i0 + n])
                    pats = []
                    for t, srcpad in enumerate((a_pad, b_pad)):
                        pt = patp.tile(
                            [128, KK], f32r, tag="pat", name=f"pt{t}_{c}"
                        )
                        for dh in range(nhj):
                            sap = bass.AP(
                                tensor=srcpad.ap().tensor,
                                offset=(h0j + dh) * PW * C,
                                ap=[
                                    [C, Wp],
                                    [PW * C, 3],
                                    [C, 3],
                                    [1, C],
                                ],
                            )
                            nc.sync.dma_start(
                                out=pt[32 * dh : 32 * (dh + 1), :],
                                in_=sap.bitcast(f32r),
                            )
                        pats.append(pt)
                    # last chunk: t-outer so tensor a's accumulators finish
                    # first and their copies/DMA overlap tensor b's matmuls
                    if c < 7:
                        order = [(p, q, t) for p in range(3) for q in range(3) for t in range(2)]
                    else:
                        order = [(p, q, t) for t in range(2) for p in range(3) for q in range(3)]
                    for p, q, t in order:
                        for cb in range(2):
                            lhs = pats[t][
                                :dm,
                                (3 * p + q) * C
                                + 128 * cb : (3 * p + q) * C
                                + 128 * (cb + 1),
                            ]
                            for hi, (i0, n, h0, nh) in enumerate(HALves):
                                rhs = t3[
                                    :dm,
                                    h0 - p + 2 : h0 - p + 2 + nh,
                                    2 - q : 2 - q + Wp,
                                ]
                                nc.tensor.matmul(
                                    yps[t][cb][hi][:, :],
                                    lhs,
                                    rhs,
                                    start=(c == 0 and p == 0 and q == 0),
                                    stop=(c == 7 and p == 2 and q == 2),
                                )

                for t, dram in enumerate((ya_t, yb_t)):
                    for cb in range(2):
                        ysb = outp.tile(
                            [128, L], f32, tag="ysb", name=f"ysb{t}_{cb}"
                        )
                        for hi, (i0, n, _, _) in enumerate(HALves):
                            nc.vector.tensor_copy(
                                ysb[:, i0 : i0 + n], yps[t][cb][hi][:, :]
                            )
                        [nc.sync, nc.scalar, nc.sync, nc.scalar][
                            2 * t + cb
                        ].dma_start(
                            out=dram[128 * cb : 128 * (cb + 1), :], in_=ysb[:]
                        )

    nc.compile()
    return nc


def _get_program():
    if "nc" not in _CACHE:
        _CACHE["nc"] = _build_program()
    return _CACHE["nc"]


def _core_inputs(A, B):
    """A, B: [31,32,256] float32 -> per-core input map."""
    ap = np.zeros((PH, PW, C), np.float32)
    ap[1 : 1 + Hp, 1 : 1 + Wp] = A
    bp = np.zeros((PH, PW, C), np.float32)
    bp[1 : 1 + Hp, 1 : 1 + Wp] = B

    def inv_norm(pad):
        s = (pad.astype(np.float64) ** 2).sum(-1)  # [33,34]
        ss = np.zeros((Hp, Wp))
        for p in range(3):
            for q in range(3):
                ss += s[p : p + Hp, q : q + Wp]
        return 1.0 / np.maximum(np.sqrt(ss), 1e-4)

    inv = (inv_norm(ap) * inv_norm(bp)).reshape(-1)  # [992]
    return {
        "a_pad": ap,
        "b_pad": bp,
        "a_chw": np.ascontiguousarray(ap.transpose(2, 0, 1).reshape(C, NPAD)),
        "b_chw": np.ascontiguousarray(bp.transpose(2, 0, 1).reshape(C, NPAD)),
        "inv_p": np.ascontiguousarray(
            np.pad(10.0 * inv, (0, 1024 - L)).reshape(8, 128).T.astype(np.float32)
        ),
        "inv_f": inv.reshape(1, L).astype(np.float32),
    }


def _untp(y_t):
    # [256, 992] channel-major -> [31, 32, 256]
    return y_t.reshape(C, Hp, Wp).transpose(1, 2, 0)


def kernel(x, mask):
    x = np.asarray(x, dtype=np.float32)
    in_maps = []
    for b in range(B_IMG):
        xb = x[b]
        in_maps.append(_core_inputs(xb[:-1], xb[1:]))
        xt = np.ascontiguousarray(xb.transpose(1, 0, 2))
        in_maps.append(_core_inputs(xt[1:], xt[:-1]))

    from concourse.bass_utils import run_bass_kernel_spmd

    nc = _get_program()
    res = run_bass_kernel_spmd(nc, in_maps, list(range(8))).results

    out = np.empty((B_IMG, H_IMG, W_IMG, C), np.float32)
    for b in range(B_IMG):
        yl = _untp(res[2 * b]["ya_t"])
        yr = _untp(res[2 * b]["yb_t"])
        ylr = np.concatenate(
            [yr[:1], (yr[1:] + yl[:-1]) * 0.5, yl[-1:]], axis=0
        )
        yt = _untp(res[2 * b + 1]["ya_t"]).transpose(1, 0, 2)
        yb = _untp(res[2 * b + 1]["yb_t"]).transpose(1, 0, 2)
        ytb = np.concatenate(
            [yt[:, :1], (yt[:, 1:] + yb[:, :-1]) * 0.5, yb[:, -1:]], axis=1
        )
        out[b] = (ylr + ytb) * 0.5
    return out



# revision 15
# speedup vs baseline: 2.9959x; 1.0669x over previous
"""EnvironmentConsistentAttention on 8 trn2 cores.

Sharding: 4 images x 2 directions (vertical/horizontal neighbor pairs) = 8
independent units, one per core. The horizontal direction of image x equals
the vertical direction of x spatially transposed, so a single SPMD program
handles both: given shifted maps A, B [31,32,256] it returns
(yA, yB) = _corr_recon(A, B), each [31,32,256] (emitted channel-major).

Per-core math (Hp=31, Wp=32, C=256, L=992, k=3):
  pa[(p,q,c), l=(h,w)] = A_pad[h+p, w+q, c]          (zero-padded patches)
  z = pa * pb                                        [2304, L]
  R = z.T @ z                                        [L, L] gram
  att[i,j] = inv[i]*inv[j]*R[i,j];  S = softmax(10*att, axis=j)
  yA = conv_transpose(S, pa) -> ya[l',c] = sum_{p,q,j} S[shift(l',p,q), j]*pa[(p,q,c), j]

att is symmetric pre-softmax, so tiles of R computed as [j-part, i-free] are
directly S.T tiles; exp/softmax-denominator (a cross-partition ones-matmul)
and the reconstruction all run in that transposed layout. S.T is stored in a
[33,34]-padded spatial grid over i so the 9 conv-transpose shifts become pure
access-pattern offsets (zero borders give SAME-padding semantics for free).
Patch norms are folded in as row/column scales of R (host precomputes the
tiny [992] inverse-norm vector).
"""

import numpy as np

Hp, Wp, C = 31, 32, 256
L = Hp * Wp            # 992
PH, PW = Hp + 2, Wp + 2  # 33, 34 padded grid
NPAD = PH * PW         # 1122
KK = 9 * C             # 2304
JC = [(128 * c, 128 if c < 7 else 96) for c in range(8)]   # j/l chunks
HALves = [(0, 512, 0, 16), (512, 480, 16, 15)]  # (i0, n, h0, nh) over i/l'
B_IMG, H_IMG, W_IMG = 4, 32, 32

_CACHE = {}


def _build_program():
    import concourse.bass as bass
    import concourse.tile as tile
    from concourse import bacc, mybir

    f32 = mybir.dt.float32
    f32r = mybir.dt.float32r
    bf16 = mybir.dt.bfloat16

    def r(ap):
        return ap.bitcast(f32r)

    nc = bacc.Bacc("TRN2", target_bir_lowering=False, debug=False)

    a_pad = nc.dram_tensor("a_pad", [PH, PW, C], bf16, kind="ExternalInput")
    b_pad = nc.dram_tensor("b_pad", [PH, PW, C], bf16, kind="ExternalInput")
    a_chw = nc.dram_tensor("a_chw", [C, NPAD], f32, kind="ExternalInput")
    b_chw = nc.dram_tensor("b_chw", [C, NPAD], f32, kind="ExternalInput")
    inv_p = nc.dram_tensor("inv_p", [128, 8], f32, kind="ExternalInput")
    inv_f = nc.dram_tensor("inv_f", [1, L], f32, kind="ExternalInput")
    ya_t = nc.dram_tensor("ya_t", [C, L], f32, kind="ExternalOutput")
    yb_t = nc.dram_tensor("yb_t", [C, L], f32, kind="ExternalOutput")

    with tile.TileContext(nc) as tc:
        from contextlib import ExitStack

        with ExitStack() as ctx:
            const = ctx.enter_context(tc.tile_pool(name="const", bufs=1))
            outp = ctx.enter_context(tc.tile_pool(name="outp", bufs=4))
            tpadp = ctx.enter_context(tc.tile_pool(name="tpad", bufs=8))

            # Constants (input DMAs for these are emitted after the chw
            # loads so the z-build critical path gets the DMA queue first)
            sb_inv_p = const.tile([128, 8], f32, tag="invp")
            sb_inv_b = const.tile([128, L], f32, tag="invb")
            ones_f = const.tile([128, 128], f32, tag="onesf")
            nc.vector.memset(ones_f[:], 1.0)
            ones_k = const.tile([128, 1], bf16, tag="onesk")
            nc.scalar.copy(ones_k[:], ones_f[:, 0:1])
            ones_m = const.tile([1, 128], bf16, tag="onesm")
            nc.scalar.copy(ones_m[:], ones_f[0:1, :])
            from concourse.masks import make_identity

            idn_f = const.tile([128, 128], f32, tag="idnf")
            idn = const.tile([128, 128], bf16, tag="idn")
            make_identity(nc, idn_f[:])
            nc.scalar.copy(idn[:], idn_f[:])
            recip_sb = const.tile([1, L], bf16, tag="recip")
            rb_sb = const.tile([128, L], bf16, tag="rbcast")

            # S.T tiles in padded-grid layout, zeroed borders
            tpad = [
                tpadp.tile([128, NPAD], bf16, tag="tpad", name=f"tpad{c}")
                for c in range(8)
            ]

            with ExitStack() as ph1:
                apadp = ph1.enter_context(tc.tile_pool(name="apad", bufs=4))
                zp = ph1.enter_context(tc.tile_pool(name="z", bufs=18))
                psD = ph1.enter_context(
                    tc.tile_pool(name="psD", bufs=1, space="PSUM")
                )

                # Load padded inputs channel-major; build z = pa*pb views
                achw, bchw = [], []
                dma_engs = [nc.sync, nc.scalar, nc.sync, nc.scalar]
                for ch in range(2):
                    ta = apadp.tile([128, NPAD], f32, tag="apad")
                    tb = apadp.tile([128, NPAD], f32, tag="apad")
                    dma_engs[2 * ch].dma_start(
                        out=ta[:], in_=a_chw[128 * ch : 128 * (ch + 1), :]
                    )
                    dma_engs[2 * ch + 1].dma_start(
                        out=tb[:], in_=b_chw[128 * ch : 128 * (ch + 1), :]
                    )
                    achw.append(ta)
                    bchw.append(tb)
                nc.sync.dma_start(out=sb_inv_p[:], in_=inv_p[:, :])
                nc.sync.dma_start(
                    out=sb_inv_b[:], in_=inv_f.ap().to_broadcast([128, L])
                )

                zt = []
                for p in range(3):
                    for q in range(3):
                        for ch in range(2):
                            k = len(zt)
                            zk = zp.tile([128, L], f32r, tag="z")
                            av = achw[ch].rearrange(
                                "c (h w) -> c h w", h=PH, w=PW
                            )[:, p : p + Hp, q : q + Wp]
                            bv = bchw[ch].rearrange(
                                "c (h w) -> c h w", h=PH, w=PW
                            )[:, p : p + Hp, q : q + Wp]
                            nc.vector.tensor_mul(zk[:], av, bv)
                            zt.append(zk)

                # zero S.T borders (gpsimd; only borders matter, interior is
                # overwritten by the exp)
                for c in range(8):
                    tf = tpad[c].rearrange(
                        "j (h w) -> j h w", h=PH, w=PW
                    )
                    nc.gpsimd.memset(tf[:, 0:1, :], 0.0)
                    nc.gpsimd.memset(tf[:, PH - 1 : PH, :], 0.0)
                    nc.gpsimd.memset(tf[:, :, 0:1], 0.0)
                    nc.gpsimd.memset(tf[:, :, PW - 1 : PW], 0.0)

                # Gram R = z.T@z per (j-chunk, i-half); scale+exp into tpad;
                # accumulate softmax denominators with ones-matmuls.
                dpsall = psD.tile([1, L], f32, tag="dps", name="dpsall")
                dps = [dpsall[:, i0 : i0 + n] for (i0, n, _, _) in HALves]
                # E is symmetric: compute only i >= 128*jc (upper block
                # triangle incl. diagonal), mirror the rest by PE transpose.
                # (i0, n, s0): matmul computes i in [i0, i0+n); only
                # [i0+s0, i0+n) is written out. All n >= 256 so f32r matmuls
                # stream at 1 cycle/row (free dims < 256 drop to 1/4 rate);
                # short tails extend left into already-covered i and skip the
                # overlap on write.
                def ichunks(jc):
                    off = 128 * jc
                    ln = L - off
                    if ln > 512:
                        n0 = ((ln + 63) // 64) * 32  # ~half, 32-aligned
                        return [(off, n0, 0), (off + n0, ln - n0, 0)]
                    if ln >= 256:
                        return [(off, ln, 0)]
                    return [(L - 256, 256, 256 - ln)]

                with tc.tile_pool(name="psR", bufs=6, space="PSUM") as psR:
                    for g0, g1 in ((0, 3), (3, 6), (6, 8)):
                        grp = list(enumerate(JC))[g0:g1]
                        rps = {
                            c: [
                                psR.tile(
                                    [128, n], f32, tag="rps", name=f"rps{c}_{ci}"
                                )
                                for ci, (i0, n, s0) in enumerate(ichunks(c))
                            ]
                            for c, _ in grp
                        }
                        # k-major so early matmuls only need early z tiles
                        for k in range(18):
                            for c, (j0, dm) in grp:
                                for ci, (i0, n, s0) in enumerate(ichunks(c)):
                                    nc.tensor.matmul(
                                        rps[c][ci][:dm, :],
                                        zt[k][:, j0 : j0 + dm],
                                        zt[k][:, i0 : i0 + n],
                                        start=(k == 0),
                                        stop=(k == 17),
                                    )
                        for c, (j0, dm) in grp:
                            t3 = tpad[c].rearrange("j (h w) -> j h w", h=PH, w=PW)
                            for ci, (i0, n, s0) in enumerate(ichunks(c)):
                                i0w, nw = i0 + s0, n - s0
                                h0, nh = i0w // Wp, nw // Wp
                                itv = t3[:dm, 1 + h0 : 1 + h0 + nh, 1 : 1 + Wp]
                                nc.vector.tensor_mul(
                                    itv,
                                    rps[c][ci][:dm, s0:n],
                                    sb_inv_b[:dm, i0w : i0w + nw],
                                )
                                nc.scalar.activation(
                                    itv,
                                    itv,
                                    mybir.ActivationFunctionType.Exp,
                                    scale=sb_inv_p[:dm, c : c + 1],
                                )

                # mirror lower-triangle blocks, then the softmax denominators
                with tc.tile_pool(name="psT", bufs=2, space="PSUM") as psT, \
                        tc.tile_pool(name="tbp", bufs=3) as tbp:
                    for c, (j0, dm) in enumerate(JC):
                        t3j = tpad[c].rearrange("j (h w) -> j h w", h=PH, w=PW)
                        nhj = dm // Wp
                        for ic in range(c):
                            t3s = tpad[ic].rearrange(
                                "j (h w) -> j h w", h=PH, w=PW
                            )
                            srcv = t3s[:128, 1 + 4 * c : 1 + 4 * c + nhj, 1 : 1 + Wp]
                            tbn = tbp.tile(
                                [128, 128], bf16, tag="tbn", name=f"tbn{c}_{ic}"
                            )
                            nc.vector.tensor_copy(tbn[:, :dm], srcv)
                            pst = psT.tile(
                                [128, 128], bf16, tag="pst", name=f"pst{c}_{ic}"
                            )
                            nc.tensor.transpose(pst[:dm, :128], tbn[:, :dm], idn[:, :])
                            nc.vector.tensor_copy(
                                t3j[:dm, 1 + 4 * ic : 1 + 4 * ic + 4, 1 : 1 + Wp],
                                pst[:dm, :128],
                            )
                        for hi, (i0, n, h0, nh) in enumerate(HALves):
                            nc.tensor.matmul(
                                dps[hi],
                                ones_k[:dm, :],
                                t3j[:dm, 1 + h0 : 1 + h0 + nh, 1 : 1 + Wp],
                                start=(c == 0),
                                stop=(c == 7),
                            )

                # 1/denom, broadcast across partitions via K=1 matmul
                rtmp2 = const.tile([1, L], f32, tag="rtmp2")
                nc.vector.reciprocal_approx_fast(out=rtmp2[:, :], in_=dpsall[:, :])
                nc.vector.tensor_copy(recip_sb[:, :], rtmp2[:, :])
                psB = ph1.enter_context(
                    tc.tile_pool(name="psB", bufs=1, space="PSUM")
                )
                bpsall = psB.tile([128, L], f32, tag="bps", name="bpsall")
                for hi, (i0, n, _, _) in enumerate(HALves):
                    nc.tensor.matmul(
                        bpsall[:, i0 : i0 + n],
                        ones_m[:, :],
                        recip_sb[:, i0 : i0 + n],
                        start=True,
                        stop=True,
                    )
                nc.scalar.copy(rb_sb[:, :], bpsall[:, :])

            # Reconstruction, a/b interleaved over one jc sweep; the
            # softmax denominator is applied to each S.T chunk at the top of
            # its jc iteration so recon matmuls chase the scaling.
            # yaT[c, l'] += sum_{p,q,j} paT[j,(p,q,c)]*S.T[j, i(l',p,q)]
            with ExitStack() as ph2:
                patp = ph2.enter_context(tc.tile_pool(name="pat", bufs=6))
                psY = ph2.enter_context(
                    tc.tile_pool(name="psY", bufs=8, space="PSUM")
                )
                yps = [
                    [
                        [
                            psY.tile(
                                [128, n], f32, tag="yps", name=f"yps{t}_{cb}_{hi}"
                            )
                            for hi, (_, n, _, _) in enumerate(HALves)
                        ]
                        for cb in range(2)
                    ]
                    for t in range(2)
                ]
                for c, (j0, dm) in enumerate(JC):
                    h0j, nhj = 4 * c, (4 if c < 7 else 3)
                    t3 = tpad[c].rearrange("j (h w) -> j h w", h=PH, w=PW)
                    for hi, (i0, n, h0, nh) in enumerate(HALves):
                        itv = t3[:dm, 1 + h0 : 1 + h0 + nh, 1 : 1 + Wp]
                        nc.vector.tensor_mul(itv, itv, rb_sb[:dm, i0 : i0 + n])
                    pats = []
                    for t, srcpad in enumerate((a_pad, b_pad)):
                        pt = patp.tile(
                            [128, KK], bf16, tag="pat", name=f"pt{t}_{c}"
                        )
                        for dh in range(nhj):
                            sap = bass.AP(
                                tensor=srcpad.ap().tensor,
                                offset=(h0j + dh) * PW * C,
                                ap=[
                                    [C, Wp],
                                    [PW * C, 3],
                                    [C, 3],
                                    [1, C],
                                ],
                            )
                            nc.sync.dma_start(
                                out=pt[32 * dh : 32 * (dh + 1), :],
                                in_=sap,
                            )
                        pats.append(pt)
                    # last chunk: t-outer so tensor a's accumulators finish
                    # first and their copies/DMA overlap tensor b's matmuls
                    if c < 7:
                        order = [(p, q, t) for p in range(3) for q in range(3) for t in range(2)]
                    else:
                        order = [(p, q, t) for t in range(2) for p in range(3) for q in range(3)]
                    for p, q, t in order:
                        for cb in range(2):
                            lhs = pats[t][
                                :dm,
                                (3 * p + q) * C
                                + 128 * cb : (3 * p + q) * C
                                + 128 * (cb + 1),
                            ]
                            for hi, (i0, n, h0, nh) in enumerate(HALves):
                                rhs = t3[
                                    :dm,
                                    h0 - p + 2 : h0 - p + 2 + nh,
                                    2 - q : 2 - q + Wp,
                                ]
                                nc.tensor.matmul(
                                    yps[t][cb][hi][:, :],
                                    lhs,
                                    rhs,
                                    start=(c == 0 and p == 0 and q == 0),
                                    stop=(c == 7 and p == 2 and q == 2),
                                )

                for t, dram in enumerate((ya_t, yb_t)):
                    for cb in range(2):
                        ysb = outp.tile(
                            [128, L], f32, tag="ysb", name=f"ysb{t}_{cb}"
                        )
                        for hi, (i0, n, _, _) in enumerate(HALves):
                            nc.vector.tensor_copy(
                                ysb[:, i0 : i0 + n], yps[t][cb][hi][:, :]
                            )
                        [nc.sync, nc.scalar, nc.sync, nc.scalar][
                            2 * t + cb
                        ].dma_start(
                            out=dram[128 * cb : 128 * (cb + 1), :], in_=ysb[:]
                        )

    nc.compile()
    return nc


def _get_program():
    if "nc" not in _CACHE:
        _CACHE["nc"] = _build_program()
    return _CACHE["nc"]


def _core_inputs(A, B):
    """A, B: [31,32,256] float32 -> per-core input map."""
    import ml_dtypes

    BF = np.dtype(ml_dtypes.bfloat16)
    ap = np.zeros((PH, PW, C), np.float32)
    ap[1 : 1 + Hp, 1 : 1 + Wp] = A
    bp = np.zeros((PH, PW, C), np.float32)
    bp[1 : 1 + Hp, 1 : 1 + Wp] = B

    def inv_norm(pad):
        s = (pad.astype(np.float64) ** 2).sum(-1)  # [33,34]
        ss = np.zeros((Hp, Wp))
        for p in range(3):
            for q in range(3):
                ss += s[p : p + Hp, q : q + Wp]
        return 1.0 / np.maximum(np.sqrt(ss), 1e-4)

    inv = (inv_norm(ap) * inv_norm(bp)).reshape(-1)  # [992]
    return {
        "a_pad": ap.astype(BF),
        "b_pad": bp.astype(BF),
        "a_chw": np.ascontiguousarray(ap.transpose(2, 0, 1).reshape(C, NPAD)),
        "b_chw": np.ascontiguousarray(bp.transpose(2, 0, 1).reshape(C, NPAD)),
        "inv_p": np.ascontiguousarray(
            np.pad(10.0 * inv, (0, 1024 - L)).reshape(8, 128).T.astype(np.float32)
        ),
        "inv_f": inv.reshape(1, L).astype(np.float32),
    }


def _untp(y_t):
    # [256, 992] channel-major -> [31, 32, 256]
    return y_t.reshape(C, Hp, Wp).transpose(1, 2, 0)


def kernel(x, mask):
    x = np.asarray(x, dtype=np.float32)
    in_maps = []
    for b in range(B_IMG):
        xb = x[b]
        in_maps.append(_core_inputs(xb[:-1], xb[1:]))
        xt = np.ascontiguousarray(xb.transpose(1, 0, 2))
        in_maps.append(_core_inputs(xt[1:], xt[:-1]))

    from concourse.bass_utils import run_bass_kernel_spmd

    nc = _get_program()
    res = run_bass_kernel_spmd(nc, in_maps, list(range(8))).results

    out = np.empty((B_IMG, H_IMG, W_IMG, C), np.float32)
    for b in range(B_IMG):
        yl = _untp(res[2 * b]["ya_t"])
        yr = _untp(res[2 * b]["yb_t"])
        ylr = np.concatenate(
            [yr[:1], (yr[1:] + yl[:-1]) * 0.5, yl[-1:]], axis=0
        )
        yt = _untp(res[2 * b + 1]["ya_t"]).transpose(1, 0, 2)
        yb = _untp(res[2 * b + 1]["yb_t"]).transpose(1, 0, 2)
        ytb = np.concatenate(
            [yt[:, :1], (yt[:, 1:] + yb[:, :-1]) * 0.5, yb[:, -1:]], axis=1
        )
        out[b] = (ylr + ytb) * 0.5
    return out



# revision 16
# speedup vs baseline: 3.0317x; 1.0119x over previous
"""EnvironmentConsistentAttention on 8 trn2 cores.

Sharding: 4 images x 2 directions (vertical/horizontal neighbor pairs) = 8
independent units, one per core. The horizontal direction of image x equals
the vertical direction of x spatially transposed, so a single SPMD program
handles both: given shifted maps A, B [31,32,256] it returns
(yA, yB) = _corr_recon(A, B), each [31,32,256] (emitted channel-major).

Per-core math (Hp=31, Wp=32, C=256, L=992, k=3):
  pa[(p,q,c), l=(h,w)] = A_pad[h+p, w+q, c]          (zero-padded patches)
  z = pa * pb                                        [2304, L]
  R = z.T @ z                                        [L, L] gram
  att[i,j] = inv[i]*inv[j]*R[i,j];  S = softmax(10*att, axis=j)
  yA = conv_transpose(S, pa) -> ya[l',c] = sum_{p,q,j} S[shift(l',p,q), j]*pa[(p,q,c), j]

att is symmetric pre-softmax, so tiles of R computed as [j-part, i-free] are
directly S.T tiles; exp/softmax-denominator (a cross-partition ones-matmul)
and the reconstruction all run in that transposed layout. S.T is stored in a
[33,34]-padded spatial grid over i so the 9 conv-transpose shifts become pure
access-pattern offsets (zero borders give SAME-padding semantics for free).
Patch norms are folded in as row/column scales of R (host precomputes the
tiny [992] inverse-norm vector).
"""

import numpy as np

Hp, Wp, C = 31, 32, 256
L = Hp * Wp            # 992
PH, PW = Hp + 2, Wp + 2  # 33, 34 padded grid
NPAD = PH * PW         # 1122
KK = 9 * C             # 2304
JC = [(128 * c, 128 if c < 7 else 96) for c in range(8)]   # j/l chunks
HALves = [(0, 512, 0, 16), (512, 480, 16, 15)]  # (i0, n, h0, nh) over i/l'
B_IMG, H_IMG, W_IMG = 4, 32, 32

_CACHE = {}


def _build_program():
    import concourse.bass as bass
    import concourse.tile as tile
    from concourse import bacc, mybir

    f32 = mybir.dt.float32
    f32r = mybir.dt.float32r
    bf16 = mybir.dt.bfloat16

    def r(ap):
        return ap.bitcast(f32r)

    nc = bacc.Bacc("TRN2", target_bir_lowering=False, debug=False)

    a_pad = nc.dram_tensor("a_pad", [PH, PW, C], bf16, kind="ExternalInput")
    b_pad = nc.dram_tensor("b_pad", [PH, PW, C], bf16, kind="ExternalInput")
    a_chw = nc.dram_tensor("a_chw", [C, NPAD], bf16, kind="ExternalInput")
    b_chw = nc.dram_tensor("b_chw", [C, NPAD], bf16, kind="ExternalInput")
    inv_p = nc.dram_tensor("inv_p", [128, 8], f32, kind="ExternalInput")
    inv_f = nc.dram_tensor("inv_f", [1, L], f32, kind="ExternalInput")
    ya_t = nc.dram_tensor("ya_t", [C, L], bf16, kind="ExternalOutput")
    yb_t = nc.dram_tensor("yb_t", [C, L], bf16, kind="ExternalOutput")

    with tile.TileContext(nc) as tc:
        from contextlib import ExitStack

        with ExitStack() as ctx:
            const = ctx.enter_context(tc.tile_pool(name="const", bufs=1))
            outp = ctx.enter_context(tc.tile_pool(name="outp", bufs=4))
            tpadp = ctx.enter_context(tc.tile_pool(name="tpad", bufs=8))

            # Constants (input DMAs for these are emitted after the chw
            # loads so the z-build critical path gets the DMA queue first)
            sb_inv_p = const.tile([128, 8], f32, tag="invp")
            sb_inv_b = const.tile([128, L], f32, tag="invb")
            ones_f = const.tile([128, 128], f32, tag="onesf")
            nc.vector.memset(ones_f[:], 1.0)
            ones_k = const.tile([128, 1], bf16, tag="onesk")
            nc.scalar.copy(ones_k[:], ones_f[:, 0:1])
            ones_m = const.tile([1, 128], bf16, tag="onesm")
            nc.scalar.copy(ones_m[:], ones_f[0:1, :])
            from concourse.masks import make_identity

            idn_f = const.tile([128, 128], f32, tag="idnf")
            idn = const.tile([128, 128], bf16, tag="idn")
            make_identity(nc, idn_f[:])
            nc.scalar.copy(idn[:], idn_f[:])
            recip_sb = const.tile([1, L], bf16, tag="recip")
            rb_sb = const.tile([128, L], bf16, tag="rbcast")

            # S.T tiles in padded-grid layout, zeroed borders
            tpad = [
                tpadp.tile([128, NPAD], bf16, tag="tpad", name=f"tpad{c}")
                for c in range(8)
            ]

            with ExitStack() as ph1:
                apadp = ph1.enter_context(tc.tile_pool(name="apad", bufs=4))
                zp = ph1.enter_context(tc.tile_pool(name="z", bufs=18))
                psD = ph1.enter_context(
                    tc.tile_pool(name="psD", bufs=1, space="PSUM")
                )

                # Load padded inputs channel-major; build z = pa*pb views
                achw, bchw = [], []
                dma_engs = [nc.sync, nc.scalar, nc.sync, nc.scalar]
                for ch in range(2):
                    ta = apadp.tile([128, NPAD], bf16, tag="apad")
                    tb = apadp.tile([128, NPAD], bf16, tag="apad")
                    dma_engs[2 * ch].dma_start(
                        out=ta[:], in_=a_chw[128 * ch : 128 * (ch + 1), :]
                    )
                    dma_engs[2 * ch + 1].dma_start(
                        out=tb[:], in_=b_chw[128 * ch : 128 * (ch + 1), :]
                    )
                    achw.append(ta)
                    bchw.append(tb)
                nc.sync.dma_start(out=sb_inv_p[:], in_=inv_p[:, :])
                nc.sync.dma_start(
                    out=sb_inv_b[:], in_=inv_f.ap().to_broadcast([128, L])
                )

                zt = []
                for p in range(3):
                    for q in range(3):
                        for ch in range(2):
                            k = len(zt)
                            zk = zp.tile([128, L], f32r, tag="z")
                            av = achw[ch].rearrange(
                                "c (h w) -> c h w", h=PH, w=PW
                            )[:, p : p + Hp, q : q + Wp]
                            bv = bchw[ch].rearrange(
                                "c (h w) -> c h w", h=PH, w=PW
                            )[:, p : p + Hp, q : q + Wp]
                            nc.vector.tensor_mul(zk[:], av, bv)
                            zt.append(zk)

                # zero S.T borders (gpsimd; only borders matter, interior is
                # overwritten by the exp)
                for c in range(8):
                    tf = tpad[c].rearrange(
                        "j (h w) -> j h w", h=PH, w=PW
                    )
                    nc.gpsimd.memset(tf[:, 0:1, :], 0.0)
                    nc.gpsimd.memset(tf[:, PH - 1 : PH, :], 0.0)
                    nc.gpsimd.memset(tf[:, :, 0:1], 0.0)
                    nc.gpsimd.memset(tf[:, :, PW - 1 : PW], 0.0)

                # Gram R = z.T@z per (j-chunk, i-half); scale+exp into tpad;
                # accumulate softmax denominators with ones-matmuls.
                dpsall = psD.tile([1, L], f32, tag="dps", name="dpsall")
                dps = [dpsall[:, i0 : i0 + n] for (i0, n, _, _) in HALves]
                # E is symmetric: compute only i >= 128*jc (upper block
                # triangle incl. diagonal), mirror the rest by PE transpose.
                # (i0, n, s0): matmul computes i in [i0, i0+n); only
                # [i0+s0, i0+n) is written out. All n >= 256 so f32r matmuls
                # stream at 1 cycle/row (free dims < 256 drop to 1/4 rate);
                # short tails extend left into already-covered i and skip the
                # overlap on write.
                def ichunks(jc):
                    off = 128 * jc
                    ln = L - off
                    if ln > 512:
                        n0 = ((ln + 63) // 64) * 32  # ~half, 32-aligned
                        return [(off, n0, 0), (off + n0, ln - n0, 0)]
                    if ln >= 256:
                        return [(off, ln, 0)]
                    return [(L - 256, 256, 256 - ln)]

                with tc.tile_pool(name="psR", bufs=6, space="PSUM") as psR:
                    for g0, g1 in ((0, 3), (3, 6), (6, 8)):
                        grp = list(enumerate(JC))[g0:g1]
                        rps = {
                            c: [
                                psR.tile(
                                    [128, n], f32, tag="rps", name=f"rps{c}_{ci}"
                                )
                                for ci, (i0, n, s0) in enumerate(ichunks(c))
                            ]
                            for c, _ in grp
                        }
                        # k-major so early matmuls only need early z tiles
                        for k in range(18):
                            for c, (j0, dm) in grp:
                                for ci, (i0, n, s0) in enumerate(ichunks(c)):
                                    nc.tensor.matmul(
                                        rps[c][ci][:dm, :],
                                        zt[k][:, j0 : j0 + dm],
                                        zt[k][:, i0 : i0 + n],
                                        start=(k == 0),
                                        stop=(k == 17),
                                    )
                        for c, (j0, dm) in grp:
                            t3 = tpad[c].rearrange("j (h w) -> j h w", h=PH, w=PW)
                            for ci, (i0, n, s0) in enumerate(ichunks(c)):
                                i0w, nw = i0 + s0, n - s0
                                h0, nh = i0w // Wp, nw // Wp
                                itv = t3[:dm, 1 + h0 : 1 + h0 + nh, 1 : 1 + Wp]
                                nc.vector.tensor_mul(
                                    itv,
                                    rps[c][ci][:dm, s0:n],
                                    sb_inv_b[:dm, i0w : i0w + nw],
                                )
                                nc.scalar.activation(
                                    itv,
                                    itv,
                                    mybir.ActivationFunctionType.Exp,
                                    scale=sb_inv_p[:dm, c : c + 1],
                                )

                # mirror lower-triangle blocks, then the softmax denominators
                with tc.tile_pool(name="psT", bufs=2, space="PSUM") as psT, \
                        tc.tile_pool(name="tbp", bufs=3) as tbp:
                    for c, (j0, dm) in enumerate(JC):
                        t3j = tpad[c].rearrange("j (h w) -> j h w", h=PH, w=PW)
                        nhj = dm // Wp
                        for ic in range(c):
                            t3s = tpad[ic].rearrange(
                                "j (h w) -> j h w", h=PH, w=PW
                            )
                            srcv = t3s[:128, 1 + 4 * c : 1 + 4 * c + nhj, 1 : 1 + Wp]
                            tbn = tbp.tile(
                                [128, 128], bf16, tag="tbn", name=f"tbn{c}_{ic}"
                            )
                            nc.vector.tensor_copy(tbn[:, :dm], srcv)
                            pst = psT.tile(
                                [128, 128], bf16, tag="pst", name=f"pst{c}_{ic}"
                            )
                            nc.tensor.transpose(pst[:dm, :128], tbn[:, :dm], idn[:, :])
                            nc.vector.tensor_copy(
                                t3j[:dm, 1 + 4 * ic : 1 + 4 * ic + 4, 1 : 1 + Wp],
                                pst[:dm, :128],
                            )
                        for hi, (i0, n, h0, nh) in enumerate(HALves):
                            nc.tensor.matmul(
                                dps[hi],
                                ones_k[:dm, :],
                                t3j[:dm, 1 + h0 : 1 + h0 + nh, 1 : 1 + Wp],
                                start=(c == 0),
                                stop=(c == 7),
                            )

                # 1/denom, broadcast across partitions via K=1 matmul
                rtmp2 = const.tile([1, L], f32, tag="rtmp2")
                nc.vector.reciprocal_approx_fast(out=rtmp2[:, :], in_=dpsall[:, :])
                nc.vector.tensor_copy(recip_sb[:, :], rtmp2[:, :])
                psB = ph1.enter_context(
                    tc.tile_pool(name="psB", bufs=1, space="PSUM")
                )
                bpsall = psB.tile([128, L], f32, tag="bps", name="bpsall")
                for hi, (i0, n, _, _) in enumerate(HALves):
                    nc.tensor.matmul(
                        bpsall[:, i0 : i0 + n],
                        ones_m[:, :],
                        recip_sb[:, i0 : i0 + n],
                        start=True,
                        stop=True,
                    )
                nc.scalar.copy(rb_sb[:, :], bpsall[:, :])

            # Reconstruction, a/b interleaved over one jc sweep; the
            # softmax denominator is applied to each S.T chunk at the top of
            # its jc iteration so recon matmuls chase the scaling.
            # yaT[c, l'] += sum_{p,q,j} paT[j,(p,q,c)]*S.T[j, i(l',p,q)]
            with ExitStack() as ph2:
                patp = ph2.enter_context(tc.tile_pool(name="pat", bufs=6))
                psY = ph2.enter_context(
                    tc.tile_pool(name="psY", bufs=8, space="PSUM")
                )
                yps = [
                    [
                        [
                            psY.tile(
                                [128, n], f32, tag="yps", name=f"yps{t}_{cb}_{hi}"
                            )
                            for hi, (_, n, _, _) in enumerate(HALves)
                        ]
                        for cb in range(2)
                    ]
                    for t in range(2)
                ]
                for c, (j0, dm) in enumerate(JC):
                    h0j, nhj = 4 * c, (4 if c < 7 else 3)
                    t3 = tpad[c].rearrange("j (h w) -> j h w", h=PH, w=PW)
                    for hi, (i0, n, h0, nh) in enumerate(HALves):
                        itv = t3[:dm, 1 + h0 : 1 + h0 + nh, 1 : 1 + Wp]
                        nc.vector.tensor_mul(itv, itv, rb_sb[:dm, i0 : i0 + n])
                    pats = []
                    for t, srcpad in enumerate((a_pad, b_pad)):
                        pt = patp.tile(
                            [128, KK], bf16, tag="pat", name=f"pt{t}_{c}"
                        )
                        for dh in range(nhj):
                            sap = bass.AP(
                                tensor=srcpad.ap().tensor,
                                offset=(h0j + dh) * PW * C,
                                ap=[
                                    [C, Wp],
                                    [PW * C, 3],
                                    [C, 3],
                                    [1, C],
                                ],
                            )
                            nc.sync.dma_start(
                                out=pt[32 * dh : 32 * (dh + 1), :],
                                in_=sap,
                            )
                        pats.append(pt)
                    # last chunk: t-outer so tensor a's accumulators finish
                    # first and their copies/DMA overlap tensor b's matmuls
                    if c < 7:
                        order = [(p, q, t) for p in range(3) for q in range(3) for t in range(2)]
                    else:
                        order = [(p, q, t) for t in range(2) for p in range(3) for q in range(3)]
                    for p, q, t in order:
                        for cb in range(2):
                            lhs = pats[t][
                                :dm,
                                (3 * p + q) * C
                                + 128 * cb : (3 * p + q) * C
                                + 128 * (cb + 1),
                            ]
                            for hi, (i0, n, h0, nh) in enumerate(HALves):
                                rhs = t3[
                                    :dm,
                                    h0 - p + 2 : h0 - p + 2 + nh,
                                    2 - q : 2 - q + Wp,
                                ]
                                nc.tensor.matmul(
                                    yps[t][cb][hi][:, :],
                                    lhs,
                                    rhs,
                                    start=(c == 0 and p == 0 and q == 0),
                                    stop=(c == 7 and p == 2 and q == 2),
                                )

                for t, dram in enumerate((ya_t, yb_t)):
                    for cb in range(2):
                        ysb = outp.tile(
                            [128, L], bf16, tag="ysb", name=f"ysb{t}_{cb}"
                        )
                        for hi, (i0, n, _, _) in enumerate(HALves):
                            nc.vector.tensor_copy(
                                ysb[:, i0 : i0 + n], yps[t][cb][hi][:, :]
                            )
                        [nc.sync, nc.scalar, nc.sync, nc.scalar][
                            2 * t + cb
                        ].dma_start(
                            out=dram[128 * cb : 128 * (cb + 1), :], in_=ysb[:]
                        )

    nc.compile()
    return nc


def _get_program():
    if "nc" not in _CACHE:
        _CACHE["nc"] = _build_program()
    return _CACHE["nc"]


def _core_inputs(A, B):
    """A, B: [31,32,256] float32 -> per-core input map."""
    import ml_dtypes

    BF = np.dtype(ml_dtypes.bfloat16)
    ap = np.zeros((PH, PW, C), np.float32)
    ap[1 : 1 + Hp, 1 : 1 + Wp] = A
    bp = np.zeros((PH, PW, C), np.float32)
    bp[1 : 1 + Hp, 1 : 1 + Wp] = B

    def inv_norm(pad):
        s = (pad.astype(np.float64) ** 2).sum(-1)  # [33,34]
        ss = np.zeros((Hp, Wp))
        for p in range(3):
            for q in range(3):
                ss += s[p : p + Hp, q : q + Wp]
        return 1.0 / np.maximum(np.sqrt(ss), 1e-4)

    inv = (inv_norm(ap) * inv_norm(bp)).reshape(-1)  # [992]
    return {
        "a_pad": ap.astype(BF),
        "b_pad": bp.astype(BF),
        "a_chw": np.ascontiguousarray(ap.transpose(2, 0, 1).reshape(C, NPAD)).astype(BF),
        "b_chw": np.ascontiguousarray(bp.transpose(2, 0, 1).reshape(C, NPAD)).astype(BF),
        "inv_p": np.ascontiguousarray(
            np.pad(10.0 * inv, (0, 1024 - L)).reshape(8, 128).T.astype(np.float32)
        ),
        "inv_f": inv.reshape(1, L).astype(np.float32),
    }


def _untp(y_t):
    # [256, 992] channel-major -> [31, 32, 256]
    return np.asarray(y_t).astype(np.float32).reshape(C, Hp, Wp).transpose(1, 2, 0)


def kernel(x, mask):
    x = np.asarray(x, dtype=np.float32)
    in_maps = []
    for b in range(B_IMG):
        xb = x[b]
        in_maps.append(_core_inputs(xb[:-1], xb[1:]))
        xt = np.ascontiguousarray(xb.transpose(1, 0, 2))
        in_maps.append(_core_inputs(xt[1:], xt[:-1]))

    from concourse.bass_utils import run_bass_kernel_spmd

    nc = _get_program()
    res = run_bass_kernel_spmd(nc, in_maps, list(range(8))).results

    out = np.empty((B_IMG, H_IMG, W_IMG, C), np.float32)
    for b in range(B_IMG):
        yl = _untp(res[2 * b]["ya_t"])
        yr = _untp(res[2 * b]["yb_t"])
        ylr = np.concatenate(
            [yr[:1], (yr[1:] + yl[:-1]) * 0.5, yl[-1:]], axis=0
        )
        yt = _untp(res[2 * b + 1]["ya_t"]).transpose(1, 0, 2)
        yb = _untp(res[2 * b + 1]["yb_t"]).transpose(1, 0, 2)
        ytb = np.concatenate(
            [yt[:, :1], (yt[:, 1:] + yb[:, :-1]) * 0.5, yb[:, -1:]], axis=1
        )
        out[b] = (ylr + ytb) * 0.5
    return out



# revision 17
# speedup vs baseline: 3.1132x; 1.0269x over previous
"""EnvironmentConsistentAttention on 8 trn2 cores.

Sharding: 4 images x 2 directions (vertical/horizontal neighbor pairs) = 8
independent units, one per core. The horizontal direction of image x equals
the vertical direction of x spatially transposed, so a single SPMD program
handles both: given shifted maps A, B [31,32,256] it returns
(yA, yB) = _corr_recon(A, B), each [31,32,256] (emitted channel-major).

Per-core math (Hp=31, Wp=32, C=256, L=992, k=3):
  pa[(p,q,c), l=(h,w)] = A_pad[h+p, w+q, c]          (zero-padded patches)
  z = pa * pb                                        [2304, L]
  R = z.T @ z                                        [L, L] gram
  att[i,j] = inv[i]*inv[j]*R[i,j];  S = softmax(10*att, axis=j)
  yA = conv_transpose(S, pa) -> ya[l',c] = sum_{p,q,j} S[shift(l',p,q), j]*pa[(p,q,c), j]

att is symmetric pre-softmax, so tiles of R computed as [j-part, i-free] are
directly S.T tiles; exp/softmax-denominator (a cross-partition ones-matmul)
and the reconstruction all run in that transposed layout. S.T is stored in a
[33,34]-padded spatial grid over i so the 9 conv-transpose shifts become pure
access-pattern offsets (zero borders give SAME-padding semantics for free).
Patch norms are folded in as row/column scales of R (host precomputes the
tiny [992] inverse-norm vector).
"""

import numpy as np

Hp, Wp, C = 31, 32, 256
L = Hp * Wp            # 992
PH, PW = Hp + 2, Wp + 2  # 33, 34 padded grid
NPAD = PH * PW         # 1122
KK = 9 * C             # 2304
JC = [(128 * c, 128 if c < 7 else 96) for c in range(8)]   # j/l chunks
HALves = [(0, 512, 0, 16), (512, 480, 16, 15)]  # (i0, n, h0, nh) over i/l'
B_IMG, H_IMG, W_IMG = 4, 32, 32

_CACHE = {}


def _build_program():
    import concourse.bass as bass
    import concourse.tile as tile
    from concourse import bacc, mybir

    f32 = mybir.dt.float32
    f32r = mybir.dt.float32r
    bf16 = mybir.dt.bfloat16

    def r(ap):
        return ap.bitcast(f32r)

    nc = bacc.Bacc("TRN2", target_bir_lowering=False, debug=False)

    a_pad = nc.dram_tensor("a_pad", [PH, PW, C], bf16, kind="ExternalInput")
    b_pad = nc.dram_tensor("b_pad", [PH, PW, C], bf16, kind="ExternalInput")
    a_chw = nc.dram_tensor("a_chw", [C, NPAD], bf16, kind="ExternalInput")
    b_chw = nc.dram_tensor("b_chw", [C, NPAD], bf16, kind="ExternalInput")
    inv_p = nc.dram_tensor("inv_p", [128, 8], f32, kind="ExternalInput")
    inv_f = nc.dram_tensor("inv_f", [1, L], f32, kind="ExternalInput")
    ya_t = nc.dram_tensor("ya_t", [C, L], bf16, kind="ExternalOutput")
    yb_t = nc.dram_tensor("yb_t", [C, L], bf16, kind="ExternalOutput")

    with tile.TileContext(nc) as tc:
        from contextlib import ExitStack

        with ExitStack() as ctx:
            const = ctx.enter_context(tc.tile_pool(name="const", bufs=1))
            outp = ctx.enter_context(tc.tile_pool(name="outp", bufs=4))
            tpadp = ctx.enter_context(tc.tile_pool(name="tpad", bufs=8))

            # Constants (input DMAs for these are emitted after the chw
            # loads so the z-build critical path gets the DMA queue first)
            sb_inv_p = const.tile([128, 8], f32, tag="invp")
            sb_inv_b = const.tile([128, L], f32, tag="invb")
            ones_f = const.tile([128, 128], f32, tag="onesf")
            nc.vector.memset(ones_f[:], 1.0)
            ones_k = const.tile([128, 1], bf16, tag="onesk")
            nc.scalar.copy(ones_k[:], ones_f[:, 0:1])
            ones_m = const.tile([1, 128], bf16, tag="onesm")
            nc.scalar.copy(ones_m[:], ones_f[0:1, :])
            from concourse.masks import make_identity

            idn_f = const.tile([128, 128], f32, tag="idnf")
            idn = const.tile([128, 128], bf16, tag="idn")
            make_identity(nc, idn_f[:])
            nc.scalar.copy(idn[:], idn_f[:])
            recip_sb = const.tile([1, L], bf16, tag="recip")
            rb_sb = const.tile([128, L], bf16, tag="rbcast")

            # S.T tiles in padded-grid layout, zeroed borders
            tpad = [
                tpadp.tile([128, NPAD], bf16, tag="tpad", name=f"tpad{c}")
                for c in range(8)
            ]

            with ExitStack() as ph1:
                apadp = ph1.enter_context(tc.tile_pool(name="apad", bufs=4))
                zp = ph1.enter_context(tc.tile_pool(name="z", bufs=18))
                psD = ph1.enter_context(
                    tc.tile_pool(name="psD", bufs=1, space="PSUM")
                )

                # Load padded inputs channel-major; build z = pa*pb views
                achw, bchw = [], []
                dma_engs = [nc.sync, nc.scalar, nc.sync, nc.scalar]
                for ch in range(2):
                    ta = apadp.tile([128, NPAD], bf16, tag="apad")
                    tb = apadp.tile([128, NPAD], bf16, tag="apad")
                    dma_engs[2 * ch].dma_start(
                        out=ta[:], in_=a_chw[128 * ch : 128 * (ch + 1), :]
                    )
                    dma_engs[2 * ch + 1].dma_start(
                        out=tb[:], in_=b_chw[128 * ch : 128 * (ch + 1), :]
                    )
                    achw.append(ta)
                    bchw.append(tb)
                nc.sync.dma_start(out=sb_inv_p[:], in_=inv_p[:, :])
                nc.sync.dma_start(
                    out=sb_inv_b[:], in_=inv_f.ap().to_broadcast([128, L])
                )

                zt = []
                for p in range(3):
                    for q in range(3):
                        for ch in range(2):
                            k = len(zt)
                            zk = zp.tile([128, L], bf16, tag="z")
                            av = achw[ch].rearrange(
                                "c (h w) -> c h w", h=PH, w=PW
                            )[:, p : p + Hp, q : q + Wp]
                            bv = bchw[ch].rearrange(
                                "c (h w) -> c h w", h=PH, w=PW
                            )[:, p : p + Hp, q : q + Wp]
                            nc.vector.tensor_mul(zk[:], av, bv)
                            zt.append(zk)

                # zero S.T borders (gpsimd; only borders matter, interior is
                # overwritten by the exp)
                for c in range(8):
                    tf = tpad[c].rearrange(
                        "j (h w) -> j h w", h=PH, w=PW
                    )
                    nc.gpsimd.memset(tf[:, 0:1, :], 0.0)
                    nc.gpsimd.memset(tf[:, PH - 1 : PH, :], 0.0)
                    nc.gpsimd.memset(tf[:, :, 0:1], 0.0)
                    nc.gpsimd.memset(tf[:, :, PW - 1 : PW], 0.0)

                # Gram R = z.T@z per (j-chunk, i-half); scale+exp into tpad;
                # accumulate softmax denominators with ones-matmuls.
                dpsall = psD.tile([1, L], f32, tag="dps", name="dpsall")
                dps = [dpsall[:, i0 : i0 + n] for (i0, n, _, _) in HALves]
                # E is symmetric: compute only i >= 128*jc (upper block
                # triangle incl. diagonal), mirror the rest by PE transpose.
                # (i0, n, s0): matmul computes i in [i0, i0+n); only
                # [i0+s0, i0+n) is written out. All n >= 256 so f32r matmuls
                # stream at 1 cycle/row (free dims < 256 drop to 1/4 rate);
                # short tails extend left into already-covered i and skip the
                # overlap on write.
                def ichunks(jc):
                    off = 128 * jc
                    ln = L - off
                    if ln > 512:
                        n0 = ((ln + 63) // 64) * 32  # ~half, 32-aligned
                        return [(off, n0, 0), (off + n0, ln - n0, 0)]
                    if ln >= 256:
                        return [(off, ln, 0)]
                    return [(L - 256, 256, 256 - ln)]

                with tc.tile_pool(name="psR", bufs=6, space="PSUM") as psR:
                    for g0, g1 in ((0, 3), (3, 6), (6, 8)):
                        grp = list(enumerate(JC))[g0:g1]
                        rps = {
                            c: [
                                psR.tile(
                                    [128, n], f32, tag="rps", name=f"rps{c}_{ci}"
                                )
                                for ci, (i0, n, s0) in enumerate(ichunks(c))
                            ]
                            for c, _ in grp
                        }
                        # k-major so early matmuls only need early z tiles
                        for k in range(18):
                            for c, (j0, dm) in grp:
                                for ci, (i0, n, s0) in enumerate(ichunks(c)):
                                    nc.tensor.matmul(
                                        rps[c][ci][:dm, :],
                                        zt[k][:, j0 : j0 + dm],
                                        zt[k][:, i0 : i0 + n],
                                        start=(k == 0),
                                        stop=(k == 17),
                                    )
                        for c, (j0, dm) in grp:
                            t3 = tpad[c].rearrange("j (h w) -> j h w", h=PH, w=PW)
                            for ci, (i0, n, s0) in enumerate(ichunks(c)):
                                i0w, nw = i0 + s0, n - s0
                                h0, nh = i0w // Wp, nw // Wp
                                itv = t3[:dm, 1 + h0 : 1 + h0 + nh, 1 : 1 + Wp]
                                nc.vector.tensor_mul(
                                    itv,
                                    rps[c][ci][:dm, s0:n],
                                    sb_inv_b[:dm, i0w : i0w + nw],
                                )
                                nc.scalar.activation(
                                    itv,
                                    itv,
                                    mybir.ActivationFunctionType.Exp,
                                    scale=sb_inv_p[:dm, c : c + 1],
                                )

                # mirror lower-triangle blocks, then the softmax denominators
                with tc.tile_pool(name="psT", bufs=2, space="PSUM") as psT, \
                        tc.tile_pool(name="tbp", bufs=3) as tbp:
                    for c, (j0, dm) in enumerate(JC):
                        t3j = tpad[c].rearrange("j (h w) -> j h w", h=PH, w=PW)
                        nhj = dm // Wp
                        for ic in range(c):
                            t3s = tpad[ic].rearrange(
                                "j (h w) -> j h w", h=PH, w=PW
                            )
                            srcv = t3s[:128, 1 + 4 * c : 1 + 4 * c + nhj, 1 : 1 + Wp]
                            tbn = tbp.tile(
                                [128, 128], bf16, tag="tbn", name=f"tbn{c}_{ic}"
                            )
                            nc.vector.tensor_copy(tbn[:, :dm], srcv)
                            pst = psT.tile(
                                [128, 128], bf16, tag="pst", name=f"pst{c}_{ic}"
                            )
                            nc.tensor.transpose(pst[:dm, :128], tbn[:, :dm], idn[:, :])
                            nc.vector.tensor_copy(
                                t3j[:dm, 1 + 4 * ic : 1 + 4 * ic + 4, 1 : 1 + Wp],
                                pst[:dm, :128],
                            )
                        for hi, (i0, n, h0, nh) in enumerate(HALves):
                            nc.tensor.matmul(
                                dps[hi],
                                ones_k[:dm, :],
                                t3j[:dm, 1 + h0 : 1 + h0 + nh, 1 : 1 + Wp],
                                start=(c == 0),
                                stop=(c == 7),
                            )

                # 1/denom, broadcast across partitions via K=1 matmul
                rtmp2 = const.tile([1, L], f32, tag="rtmp2")
                nc.vector.reciprocal_approx_fast(out=rtmp2[:, :], in_=dpsall[:, :])
                nc.vector.tensor_copy(recip_sb[:, :], rtmp2[:, :])
                psB = ph1.enter_context(
                    tc.tile_pool(name="psB", bufs=1, space="PSUM")
                )
                bpsall = psB.tile([128, L], f32, tag="bps", name="bpsall")
                for hi, (i0, n, _, _) in enumerate(HALves):
                    nc.tensor.matmul(
                        bpsall[:, i0 : i0 + n],
                        ones_m[:, :],
                        recip_sb[:, i0 : i0 + n],
                        start=True,
                        stop=True,
                    )
                nc.scalar.copy(rb_sb[:, :], bpsall[:, :])

            # Reconstruction, a/b interleaved over one jc sweep; the
            # softmax denominator is applied to each S.T chunk at the top of
            # its jc iteration so recon matmuls chase the scaling.
            # yaT[c, l'] += sum_{p,q,j} paT[j,(p,q,c)]*S.T[j, i(l',p,q)]
            with ExitStack() as ph2:
                patp = ph2.enter_context(tc.tile_pool(name="pat", bufs=6))
                psY = ph2.enter_context(
                    tc.tile_pool(name="psY", bufs=8, space="PSUM")
                )
                yps = [
                    [
                        [
                            psY.tile(
                                [128, n], f32, tag="yps", name=f"yps{t}_{cb}_{hi}"
                            )
                            for hi, (_, n, _, _) in enumerate(HALves)
                        ]
                        for cb in range(2)
                    ]
                    for t in range(2)
                ]
                for c, (j0, dm) in enumerate(JC):
                    h0j, nhj = 4 * c, (4 if c < 7 else 3)
                    t3 = tpad[c].rearrange("j (h w) -> j h w", h=PH, w=PW)
                    for hi, (i0, n, h0, nh) in enumerate(HALves):
                        itv = t3[:dm, 1 + h0 : 1 + h0 + nh, 1 : 1 + Wp]
                        nc.vector.tensor_mul(itv, itv, rb_sb[:dm, i0 : i0 + n])
                    pats = []
                    for t, srcpad in enumerate((a_pad, b_pad)):
                        pt = patp.tile(
                            [128, KK], bf16, tag="pat", name=f"pt{t}_{c}"
                        )
                        for dh in range(nhj):
                            sap = bass.AP(
                                tensor=srcpad.ap().tensor,
                                offset=(h0j + dh) * PW * C,
                                ap=[
                                    [C, Wp],
                                    [PW * C, 3],
                                    [C, 3],
                                    [1, C],
                                ],
                            )
                            nc.sync.dma_start(
                                out=pt[32 * dh : 32 * (dh + 1), :],
                                in_=sap,
                            )
                        pats.append(pt)
                    # last chunk: t-outer so tensor a's accumulators finish
                    # first and their copies/DMA overlap tensor b's matmuls
                    if c < 7:
                        order = [(p, q, t) for p in range(3) for q in range(3) for t in range(2)]
                    else:
                        order = [(p, q, t) for t in range(2) for p in range(3) for q in range(3)]
                    for p, q, t in order:
                        for cb in range(2):
                            lhs = pats[t][
                                :dm,
                                (3 * p + q) * C
                                + 128 * cb : (3 * p + q) * C
                                + 128 * (cb + 1),
                            ]
                            for hi, (i0, n, h0, nh) in enumerate(HALves):
                                rhs = t3[
                                    :dm,
                                    h0 - p + 2 : h0 - p + 2 + nh,
                                    2 - q : 2 - q + Wp,
                                ]
                                nc.tensor.matmul(
                                    yps[t][cb][hi][:, :],
                                    lhs,
                                    rhs,
                                    start=(c == 0 and p == 0 and q == 0),
                                    stop=(c == 7 and p == 2 and q == 2),
                                )

                for t, dram in enumerate((ya_t, yb_t)):
                    for cb in range(2):
                        ysb = outp.tile(
                            [128, L], bf16, tag="ysb", name=f"ysb{t}_{cb}"
                        )
                        for hi, (i0, n, _, _) in enumerate(HALves):
                            nc.scalar.copy(
                                ysb[:, i0 : i0 + n], yps[t][cb][hi][:, :]
                            )
                        [nc.sync, nc.scalar, nc.sync, nc.scalar][
                            2 * t + cb
                        ].dma_start(
                            out=dram[128 * cb : 128 * (cb + 1), :], in_=ysb[:]
                        )

    nc.compile()
    return nc


def _get_program():
    if "nc" not in _CACHE:
        _CACHE["nc"] = _build_program()
    return _CACHE["nc"]


def _core_inputs(A, B):
    """A, B: [31,32,256] float32 -> per-core input map."""
    import ml_dtypes

    BF = np.dtype(ml_dtypes.bfloat16)
    ap = np.zeros((PH, PW, C), np.float32)
    ap[1 : 1 + Hp, 1 : 1 + Wp] = A
    bp = np.zeros((PH, PW, C), np.float32)
    bp[1 : 1 + Hp, 1 : 1 + Wp] = B

    def inv_norm(pad):
        s = (pad.astype(np.float64) ** 2).sum(-1)  # [33,34]
        ss = np.zeros((Hp, Wp))
        for p in range(3):
            for q in range(3):
                ss += s[p : p + Hp, q : q + Wp]
        return 1.0 / np.maximum(np.sqrt(ss), 1e-4)

    inv = (inv_norm(ap) * inv_norm(bp)).reshape(-1)  # [992]
    return {
        "a_pad": ap.astype(BF),
        "b_pad": bp.astype(BF),
        "a_chw": np.ascontiguousarray(ap.transpose(2, 0, 1).reshape(C, NPAD)).astype(BF),
        "b_chw": np.ascontiguousarray(bp.transpose(2, 0, 1).reshape(C, NPAD)).astype(BF),
        "inv_p": np.ascontiguousarray(
            np.pad(10.0 * inv, (0, 1024 - L)).reshape(8, 128).T.astype(np.float32)
        ),
        "inv_f": inv.reshape(1, L).astype(np.float32),
    }


def _untp(y_t):
    # [256, 992] channel-major -> [31, 32, 256]
    return np.asarray(y_t).astype(np.float32).reshape(C, Hp, Wp).transpose(1, 2, 0)


def kernel(x, mask):
    x = np.asarray(x, dtype=np.float32)
    in_maps = []
    for b in range(B_IMG):
        xb = x[b]
        in_maps.append(_core_inputs(xb[:-1], xb[1:]))
        xt = np.ascontiguousarray(xb.transpose(1, 0, 2))
        in_maps.append(_core_inputs(xt[1:], xt[:-1]))

    from concourse.bass_utils import run_bass_kernel_spmd

    nc = _get_program()
    res = run_bass_kernel_spmd(nc, in_maps, list(range(8))).results

    out = np.empty((B_IMG, H_IMG, W_IMG, C), np.float32)
    for b in range(B_IMG):
        yl = _untp(res[2 * b]["ya_t"])
        yr = _untp(res[2 * b]["yb_t"])
        ylr = np.concatenate(
            [yr[:1], (yr[1:] + yl[:-1]) * 0.5, yl[-1:]], axis=0
        )
        yt = _untp(res[2 * b + 1]["ya_t"]).transpose(1, 0, 2)
        yb = _untp(res[2 * b + 1]["yb_t"]).transpose(1, 0, 2)
        ytb = np.concatenate(
            [yt[:, :1], (yt[:, 1:] + yb[:, :-1]) * 0.5, yb[:, -1:]], axis=1
        )
        out[b] = (ylr + ytb) * 0.5
    return out

